# revision 1
# baseline (speedup 1.0000x reference)
"""AMMLinear (vq_codebook) forward kernel for 8 TRN2 NeuronCores.

Key algebraic fact: the reference's straight-through estimator
    output = real - stop_grad(real - quantized)
is numerically exactly `quantized_output + bias`, so the forward value needs
only:  argmin-distance one-hot  @  fake-quantized lut  + bias.
The softmax/attention path contributes gradients only.

Distribution: data-parallel over the 8192 tokens (1024/core); the lut
(= centroids @ weight, then int8 fake-quant) is computed sharded over
out_features (512 columns/core) and allgathered as exact-int bf16 `q`,
with the global quant scale obtained via a 4-byte AllReduce(max).

Per-core device pipeline:
  L: lut slice (block-diag matmul trick, full 128-contraction), |.|max,
     AllReduce scale, quantize q = round(lut/scale) via the fp32 +1.5*2^23
     round-to-nearest-even trick, exact small ints stored in bf16.
  S: scores e - 0.5*c2 per codebook (block-diag matmul, fp32), argmax over
     the 16 centroids -> first-index one-hot in [codebook*16+k, token]
     layout (exact integer compares; tie-safe).
  G: out.T[o_tile, tokens] += sum_g onehot_g.T-weighted q columns as
     dense 128-contraction bf16 matmuls accumulated in PSUM, epilogue
     Identity(psum*scale + bias_col) on ScalarE, contiguous DMA out.
Host gathers the per-core out.T shards and transposes (layout only).
"""

import numpy as np

N_TOKENS = 8192
IN_FEAT = 1024
C = 64  # codebooks
KC = 16  # centroids per codebook
S = 16  # subvector length
O = 4096  # out features
NCORES = 8
NLOC = N_TOKENS // NCORES  # 1024 tokens per core
G = 8  # groups of 8 codebooks -> 128-row contraction
OSL = O // NCORES  # 512-wide lut o-slice per core
TT = NLOC // 128  # 8 token tiles
OTILES = O // 128  # 32 o tiles
MAGIC = 12582912.0  # 1.5 * 2^23: fp32 add => round-to-nearest-even integer
BIG = 4096.0

_CACHED = {}


def _consts():
    kiota = (79.0 - np.arange(128, dtype=np.float32) % KC).reshape(128, 1)
    iotabig = np.tile(
        15.0 - (np.arange(1024, dtype=np.float32) % KC), (128, 1)
    ).astype(np.float32)
    ident = np.eye(128, dtype=np.float32)
    onescol = np.ones((128, 1), np.float32)
    onesrow = np.ones((1, 128), np.float32)
    return kiota, iotabig, ident, onescol, onesrow


def build_nc():
    import concourse.bacc as bacc
    import concourse.mybir as mybir
    import concourse.tile as tile
    import concourse.bass_isa as bass_isa
    from contextlib import ExitStack

    f32 = mybir.dt.float32
    bf16 = mybir.dt.bfloat16
    AO = mybir.AluOpType
    AF = mybir.ActivationFunctionType
    X = mybir.AxisListType.X

    nc = bacc.Bacc(
        "TRN2", target_bir_lowering=False, debug=False, num_devices=NCORES
    )

    xt_d = nc.dram_tensor("xt", [128, G, NLOC], f32, kind="ExternalInput")
    bd_d = nc.dram_tensor("bd", [128, G, 128], f32, kind="ExternalInput")
    wsl_d = nc.dram_tensor("wsl", [128, G, OSL], f32, kind="ExternalInput")
    biasT_d = nc.dram_tensor("biasT", [128, OTILES], f32, kind="ExternalInput")
    kiota_d = nc.dram_tensor("kiota", [128, 1], f32, kind="ExternalInput")
    iob_d = nc.dram_tensor("iotabig", [128, 1024], f32, kind="ExternalInput")
    id_d = nc.dram_tensor("ident", [128, 128], f32, kind="ExternalInput")
    oc_d = nc.dram_tensor("onescol", [128, 1], f32, kind="ExternalInput")
    or_d = nc.dram_tensor("onesrow", [1, 128], f32, kind="ExternalInput")
    out_d = nc.dram_tensor("out", [O, NLOC], f32, kind="ExternalOutput")

    groups = [list(range(NCORES))]

    with ExitStack() as ctx:
        tc = ctx.enter_context(tile.TileContext(nc))
        sb = ctx.enter_context(tc.tile_pool(name="sb", bufs=1))
        sbw = ctx.enter_context(tc.tile_pool(name="sbw", bufs=2))
        sbo = ctx.enter_context(tc.tile_pool(name="sbo", bufs=3))
        psA = ctx.enter_context(tc.tile_pool(name="psA", bufs=3, space="PSUM"))
        psB = ctx.enter_context(tc.tile_pool(name="psB", bufs=2, space="PSUM"))
        dram = ctx.enter_context(tc.tile_pool(name="dram", bufs=1, space="DRAM"))
        i8 = mybir.dt.int8

        # ---------- persistent SBUF tensors ----------
        bd_sb = sb.tile([128, G, 128], f32)
        # wsl and q_bf share a slot: wsl's last read (lut matmul) precedes
        # the arrival of the allgathered q.
        wsl_sb = sb.tile([128, G, OSL], f32, tag="bigA")
        lut_sb = sb.tile([128, G, OSL], f32)
        xt_sb = sb.tile([128, G, NLOC], f32)
        oh_sb = sb.tile([128, G, NLOC], bf16)
        biasT_sb = sb.tile([128, OTILES], f32)
        kiota2_sb = sb.tile([128, 1], f32)
        ioneg_sb = sb.tile([128, 1024], f32)
        id_sb = sb.tile([128, 128], f32)
        oc_sb = sb.tile([128, 1], f32)
        or_sb = sb.tile([1, 128], f32)
        nc2_sb = sb.tile([1, 1024], f32)
        idxT_sb = sb.tile([64, NLOC], bf16)
        mg_sb = sb.tile([128, G], f32)
        m1_sb = sb.tile([128, 1], f32)
        m2_sb = sb.tile([128, 1], f32)
        mrow_sb = sb.tile([1, 128], f32)
        mglob_sb = sb.tile([1, 1], f32)
        mcol_sb = sb.tile([128, 1], f32)
        rec_sb = sb.tile([128, 1], f32)
        inv_sb = sb.tile([128, 1], f32)
        scale_sb = sb.tile([128, 1], f32)
        magic_sb = sb.tile([128, 1], f32)
        negmagic_sb = sb.tile([128, 1], f32)
        kiota2b_sb = sb.tile([128, 1], bf16)

        # ---------- lut-chain inputs first: they gate the scale AllReduce,
        # the longest-latency item in the prologue ----------
        for g in range(G):
            nc.sync.dma_start(wsl_sb[:, g, :], wsl_d[:, g, :])
        for g in range(0, G, 2):
            nc.sync.dma_start(bd_sb[:, g : g + 2, :], bd_d[:, g : g + 2, :])
        nc.scalar.dma_start(id_sb[:], id_d[:])
        nc.scalar.dma_start(oc_sb[:], oc_d[:])
        nc.scalar.dma_start(or_sb[:], or_d[:])
        nc.scalar.dma_start(biasT_sb[:], biasT_d[:])
        nc.scalar.dma_start(kiota2_sb[:], kiota_d[:])
        nc.scalar.dma_start(ioneg_sb[:], iob_d[:])
        nc.vector.memset(magic_sb[:], MAGIC)
        nc.vector.memset(negmagic_sb[:], -MAGIC)

        # ---------- phase L: lut slice, |.|max, scale AllReduce ----------
        for g in range(G):
            lut_ps = psB.tile([128, OSL], f32, tag="w1", name=f"lut_ps{g}")
            nc.tensor.matmul(
                lut_ps[:], bd_sb[:, g, :], wsl_sb[:, g, :], start=True, stop=True
            )
            nc.vector.tensor_reduce(
                mg_sb[:, g : g + 1], lut_ps[:], axis=X, op=AO.max,
                apply_absolute_value=True,
            )
            nc.scalar.copy(lut_sb[:, g, :], lut_ps[:])
        nc.vector.tensor_reduce(m1_sb[:], mg_sb[:], axis=X, op=AO.max)
        # cross-partition max: transpose (128,1)->(1,128), reduce, AllReduce
        # across cores, then broadcast back to 128 partitions via matmul.
        mt_ps = psB.tile([1, 128], f32, tag="w1", name="mt_ps")
        nc.tensor.transpose(mt_ps[:], m1_sb[:], id_sb[:])
        nc.scalar.copy(mrow_sb[:], mt_ps[:])
        nc.vector.tensor_reduce(m2_sb[0:1, :], mrow_sb[:], axis=X, op=AO.max)
        m_in_d = dram.tile([1, 1], f32)
        m_out_d = dram.tile([1, 1], f32, addr_space="Shared")
        nc.sync.dma_start(m_in_d[:], m2_sb[0:1, 0:1])
        nc.gpsimd.collective_compute(
            "AllReduce", AO.max, replica_groups=groups,
            ins=[m_in_d.opt()], outs=[m_out_d.opt()],
        )
        nc.sync.dma_start(mglob_sb[:], m_out_d[:])

        # x arrives behind the lut-chain traffic by construction
        for g in range(G):
            nc.sync.dma_start(xt_sb[:, g, :], xt_d[:, g, :])

        # ---------- c2 = sum_s bd^2 per ck ----------
        sq_sb = sbw.tile([128, G, 128], f32, tag="sq", bufs=1)
        nc.scalar.square(sq_sb[:], bd_sb[:])
        nc.vector.tensor_copy(kiota2b_sb[:], kiota2_sb[:])
        c2_ps = psA.tile([1, 1024], f32, tag="w2", name="c2_ps")
        for g in range(G):
            nc.tensor.matmul(
                c2_ps[:, g * 128 : (g + 1) * 128], oc_sb[:], sq_sb[:, g, :],
                start=True, stop=True,
            )
        nc.vector.tensor_scalar_mul(nc2_sb[:], c2_ps[:], -0.5)

        # ---------- phase S: scores -> first-max one-hot ----------
        def emit_tile(t):
            tok = slice(t * 128, (t + 1) * 128)
            sc_ps = psA.tile([128, 1024], f32, tag="w2", name=f"sc_ps{t}")
            for h in range(2):
                nc.tensor.matmul(
                    sc_ps[:, h * 512 : (h + 1) * 512], or_sb[:],
                    nc2_sb[:, h * 512 : (h + 1) * 512],
                    start=True, stop=False, skip_group_check=True,
                )
            for g in range(G):
                # banks: cols [0:512] = groups 0-3, [512:1024] = groups 4-7
                nc.tensor.matmul(
                    sc_ps[:, g * 128 : (g + 1) * 128],
                    xt_sb[:, g, tok], bd_sb[:, g, :],
                    start=False, stop=(g % 4 == 3), skip_group_check=True,
                )
            maxb = sbw.tile([128, C], f32, tag="maxb", name=f"maxb{t}")
            nc.vector.tensor_reduce(
                maxb[:], sc_ps[:].rearrange("p (c k) -> p c k", k=KC),
                axis=X, op=AO.max,
            )
            mask = sbw.tile([128, 1024], f32, tag="mask", name=f"mask{t}")
            nc.vector.tensor_tensor(
                mask[:].rearrange("p (c k) -> p c k", k=KC),
                sc_ps[:].rearrange("p (c k) -> p c k", k=KC),
                maxb[:].rearrange("p (c u) -> p c u", u=1).broadcast_to((128, C, KC)),
                op=AO.is_equal,
            )
            # iv = mask*64 + (15-k): max picks the first (smallest-k) hit,
            # encoded as 64+15-k (exact in bf16 downstream).
            nc.vector.scalar_tensor_tensor(
                mask[:], mask[:], 64.0, ioneg_sb[:], op0=AO.mult, op1=AO.add
            )
            idxt = sbw.tile([128, C], f32, tag="idxt", name=f"idxt{t}")
            nc.vector.tensor_reduce(
                idxt[:], mask[:].rearrange("p (c k) -> p c k", k=KC),
                axis=X, op=AO.max,
            )
            tp_ps = psB.tile([64, 128], f32, tag="w1", name=f"tp_ps{t}")
            nc.tensor.transpose(tp_ps[:], idxt[:], id_sb[:])
            nc.scalar.copy(idxT_sb[:, tok], tp_ps[:])

        for t in range(5):
            emit_tile(t)

        # ---------- scale consume + quantize + q AllGather (mid-S so the
        # engine streams reach these only after the AllReduce has landed) --
        mc_ps = psB.tile([128, 1], f32, tag="w1", name="mc_ps")
        nc.tensor.matmul(mc_ps[:], or_sb[:], mglob_sb[:], start=True, stop=True)
        nc.scalar.copy(mcol_sb[:], mc_ps[:])
        nc.vector.reciprocal(rec_sb[:], mcol_sb[:])
        nc.vector.tensor_scalar_mul(inv_sb[:], rec_sb[:], 127.0)
        nc.vector.tensor_scalar_mul(scale_sb[:], mcol_sb[:], 1.0 / 127.0)
        q_own = sb.tile([128, G, OSL], i8)
        for g in range(G):
            t_g = sbw.tile([128, OSL], f32, tag="tg", name=f"tg{g}")
            # t = round_to_int(lut * (127/max)) + MAGIC   (fp32 RNE trick)
            nc.vector.scalar_tensor_tensor(
                t_g[:], lut_sb[:, g, :], inv_sb[:, 0:1],
                magic_sb[:, 0:1].broadcast_to((128, OSL)),
                op0=AO.mult, op1=AO.add,
            )
            # q = t - MAGIC: exact small ints, shipped as int8
            nc.scalar.activation(
                q_own[:, g, :], t_g[:], AF.Identity,
                bias=negmagic_sb[:, 0:1], scale=1.0,
            )
        H = OSL // 2
        q_in_A = dram.tile([128, G, H], i8)
        q_in_B = dram.tile([128, G, H], i8)
        q_out_A = dram.tile([NCORES, 128, G, H], i8, addr_space="Shared")
        q_out_B = dram.tile([NCORES, 128, G, H], i8, addr_space="Shared")
        nc.sync.dma_start(q_in_A[:], q_own[:, :, 0:H])
        nc.gpsimd.collective_compute(
            "AllGather", AO.bypass, replica_groups=groups,
            ins=[q_in_A.opt()], outs=[q_out_A.opt()],
        )
        nc.sync.dma_start(q_in_B[:], q_own[:, :, H:OSL])
        nc.gpsimd.collective_compute(
            "AllGather", AO.bypass, replica_groups=groups,
            ins=[q_in_B.opt()], outs=[q_out_B.opt()],
        )
        for t in range(5, TT):
            emit_tile(t)

        # expand idx over the 16 centroid slots: idxb[16j+k, n] = idxT[8g+j, n]
        for g in range(G):
            idxb = sbw.tile([128, NLOC], bf16, tag="idxb", name=f"idxb{g}")
            nc.scalar.dma_start(
                idxb[:],
                idxT_sb[g * 8 : (g + 1) * 8, :]
                .rearrange("j (n u) -> j u n", u=1)
                .broadcast_to((8, KC, NLOC)),
            )
            nc.vector.tensor_tensor(
                oh_sb[:, g, :], idxb[:],
                kiota2b_sb[:, 0:1].broadcast_to((128, NLOC)),
                op=AO.is_equal,
            )

        q_bfs = [
            sb.tile([128, G, OSL], bf16, tag=f"qbf{r}", name=f"qbf{r}")
            for r in range(NCORES)
        ]
        for hi, q_out_h in ((0, q_out_A), (1, q_out_B)):
            for r in range(NCORES):
                q_i8 = sbw.tile([128, G, H], i8, tag="qi8", name=f"qi8_{hi}_{r}")
                nc.sync.dma_start(q_i8[:], q_out_h[r])
                nc.vector.tensor_copy(
                    q_bfs[r][:, :, hi * H : (hi + 1) * H], q_i8[:]
                )

        # ---------- phase G: gather matmuls + epilogue ----------
        # half-A o-tiles (cols 0:256 of each rank) first: they only need
        # the first AllGather; half B streams in behind them.
        ot_order = [4 * r + s for r in range(NCORES) for s in (0, 1)]
        ot_order += [4 * r + s for r in range(NCORES) for s in (2, 3)]
        for ot in ot_order:
            r, osub = divmod(ot, OSL // 128)  # owning rank, 128-col offset
            osub *= 128
            gat_ps = psA.tile([128, NLOC], f32, tag="w2", name=f"gat{ot}")
            for g in range(G):
                for h in range(2):
                    nc.tensor.matmul(
                        gat_ps[:, h * 512 : (h + 1) * 512],
                        q_bfs[r][:, g, osub : osub + 128],
                        oh_sb[:, g, h * 512 : (h + 1) * 512],
                        start=(g == 0), stop=(g == G - 1),
                        skip_group_check=True,
                    )
            o_sb = sbo.tile([128, NLOC], f32, tag="osb", name=f"osb{ot}")
            nc.scalar.activation(
                o_sb[:], gat_ps[:], AF.Identity,
                bias=biasT_sb[:, ot : ot + 1], scale=scale_sb[:, 0:1],
            )
            nc.sync.dma_start(out_d[ot * 128 : (ot + 1) * 128, :], o_sb[:])

    nc.compile()
    return nc


def _prep_inputs(x, centroids, weight, bias):
    """Host-side shard/layout prep (pure data movement + constants)."""
    kiota, iotabig, ident, onescol, onesrow = _consts()
    # block-diagonal centroids^T: bd[s, g, ck];  block j of group g is
    # centroids[8g+j].T  (S x K)
    bd = np.zeros((128, G, 128), np.float32)
    for g in range(G):
        for j in range(8):
            bd[16 * j : 16 * (j + 1), g, 16 * j : 16 * (j + 1)] = centroids[
                8 * g + j
            ].T
    wflat = np.ascontiguousarray(weight.reshape(C * S, O))  # [128g+p, o]
    biasT = np.ascontiguousarray(bias.reshape(OTILES, 128).T)
    common = dict(
        bd=bd, biasT=biasT, kiota=kiota, iotabig=iotabig, ident=ident,
        onescol=onescol, onesrow=onesrow,
    )
    in_maps = []
    for i in range(NCORES):
        xs = x[i * NLOC : (i + 1) * NLOC, :]  # (1024, 1024)
        xt = np.ascontiguousarray(
            xs.T.reshape(G, 128, NLOC).transpose(1, 0, 2)
        )  # [p, g, n]
        wsl = np.ascontiguousarray(
            wflat[:, i * OSL : (i + 1) * OSL].reshape(G, 128, OSL).transpose(1, 0, 2)
        )  # [p, g, o']
        m = dict(common)
        m.update(xt=xt, wsl=wsl)
        in_maps.append({k: np.ascontiguousarray(v) for k, v in m.items()})
    return in_maps


def kernel(x, centroids, weight, inverse_temperature_logit, bias, **_):
    from concourse.bass_utils import run_bass_kernel_spmd

    x = np.asarray(x, np.float32)
    centroids = np.asarray(centroids, np.float32)
    weight = np.asarray(weight, np.float32)
    bias = np.asarray(bias, np.float32)

    if "nc" not in _CACHED:
        _CACHED["nc"] = build_nc()
    nc = _CACHED["nc"]

    in_maps = _prep_inputs(x, centroids, weight, bias)
    res = run_bass_kernel_spmd(nc, in_maps, core_ids=list(range(NCORES)))
    out = np.empty((N_TOKENS, O), np.float32)
    for i in range(NCORES):
        out[i * NLOC : (i + 1) * NLOC, :] = res.results[i]["out"].T
    return out

